# revision 19
# baseline (speedup 1.0000x reference)
"""Causal MHA (B=2, S=2048, E=2048, H=16, Dh=128) on 8 trn2 NeuronCores.

Sharding (megatron-style, all host-side): core c -> batch b=c//4, head-group
g=c%4 (4 heads). Each core computes Q/K/V for its 4 heads from the full x[b],
causal attention, and a partial output projection y_partial = attn_out @
Wo[rows of g]. Host sums the 4 partials per batch (+bo) and reassembles the
kv cache. All 8 cores run one identical SPMD graph; no collectives.

Device layout trick: x is fed pre-transposed [E, S] so the projections
produce Q^T/K^T per head ([d=128 partitions, s free]) and V natural [s, d]
directly. Scores are computed transposed ([k, q]); exp runs on ScalarE with
the 1/sqrt(Dh) scale folded in; the causal mask is a 0/1 multiply; the
softmax denominator is a ones-matmul on the TensorEngine (partition-dim sum,
replicated on all partitions); attn@V consumes exp^T as the matmul rhs with
V as lhsT; the output projection consumes attn_out^T as lhsT. Zero on-chip
transposes.
"""

import sys

import numpy as np

try:
    import concourse.bass as bass
except ImportError:
    sys.path.insert(0, "/opt/trn_rl_repo")
    import concourse.bass as bass

import bass_rust
import ml_dtypes
import concourse.mybir as mybir
import concourse.tile as tile
from concourse.vector_clock import ScopedClock
from concourse.bass_utils import run_bass_kernel_spmd

BF16 = ml_dtypes.bfloat16
B, S, E, H = 2, 2048, 2048, 16
DH = 128
P = 128
D = 512  # per-core head-dim slice (4 heads)
HL = 4  # local heads
NE = E // P  # 16 e-tiles
NC = S // 512  # 4 s/q chunks
SCALE = 1.0 / float(np.sqrt(DH))
FP32 = mybir.dt.float32
BF = mybir.dt.bfloat16


def _patched_drain_and_barrier(self, tick_clock, wait_clock):
    # Walrus CoreV3 lowers Drain to a TPB_CTRL encoding with a single
    # sync-wait slot; Tile's stock exit puts every outstanding proc's wait on
    # one drain and the compile fails with "Too many sync wait commands".
    # Keep one wait on the drain and chain the rest through SP nops.
    drain_inst = self.nc.sync.drain()
    wait_clock.add_sem_waits(
        drain_inst.ins, ScopedClock({None: tick_clock.global_clock})
    )
    si = drain_inst.ins.sync_info
    waits = list(si.on_wait) if si is not None else []
    if len(waits) > 1:
        si.on_wait = waits[:1]
        drain_inst.ins.sync_info = si
        for w in waits[1:]:
            extra = self.nc.sync.nop(nofuse=True, hint="drain_wait_chain")
            extra.ins.sync_info = bass_rust.SyncInfo(on_wait=[w], on_update=[])

    self.nc.all_engine_barrier()
    assert self.sems is not None
    popped = self.nc._tile_sem_poison_stack.pop()
    assert popped is self._sem_poison
    self.nc.clear_and_free_semaphores(list(self.sems.allocated().values()))
    self.nc.all_engine_barrier()


tile.TileContext._drain_and_barrier = _patched_drain_and_barrier

_orig_lower_ordered = tile.TileContext._lower_ordered_insts
_split_ctr = [0]


def _split_waits_lower(self, ordered):
    # Same walrus limitation as above, but for body instructions: each
    # instruction may carry at most one sync wait. Hoist extras onto
    # same-engine NoOps placed immediately before the instruction.
    for bb_name, insts in ordered.items():
        new = []
        for inst in insts:
            si = inst.sync_info
            if si is not None and len(si.on_wait) > 1:
                waits = list(si.on_wait)
                for w in waits[:-1]:
                    _split_ctr[0] += 1
                    nop = mybir.InstNoOp(name=f"waitsplit-{_split_ctr[0]}")
                    nop.engine = inst.engine
                    nop.sync_info = bass_rust.SyncInfo(on_wait=[w], on_update=[])
                    new.append(nop)
                si.on_wait = waits[-1:]
                inst.sync_info = si
            new.append(inst)
        ordered[bb_name] = new
    return _orig_lower_ordered(self, ordered)


tile.TileContext._lower_ordered_insts = _split_waits_lower

AF = mybir.ActivationFunctionType


def build_graph():
    nc = bass.Bass(trn_type="TRN2")

    xT = nc.dram_tensor("xT", [E, S], BF, kind="ExternalInput")
    wq = nc.dram_tensor("wq", [E, D], BF, kind="ExternalInput")
    wk = nc.dram_tensor("wk", [E, D], BF, kind="ExternalInput")
    wv = nc.dram_tensor("wv", [E, D], BF, kind="ExternalInput")
    wo = nc.dram_tensor("wo", [D, E], BF, kind="ExternalInput")
    bqk = nc.dram_tensor("bqk", [P, 2 * HL], FP32, kind="ExternalInput")
    bvr = nc.dram_tensor("bvr", [P, D], FP32, kind="ExternalInput")
    maskT = nc.dram_tensor("maskT", [4, P, 512], BF, kind="ExternalInput")

    y_out = nc.dram_tensor("y", [S, E], FP32, kind="ExternalOutput")
    kT_out = nc.dram_tensor("kT", [D, S], BF, kind="ExternalOutput")
    v_out = nc.dram_tensor("v", [S, D], BF, kind="ExternalOutput")

    with tile.TileContext(nc) as tc:
        with (
            tc.tile_pool(name="const", bufs=1) as cpool,
            tc.tile_pool(name="persist", bufs=1) as ppool,
            tc.tile_pool(name="qt", bufs=2) as qtpool,
            tc.tile_pool(name="aot", bufs=2) as aotpool,
            tc.tile_pool(name="xs", bufs=20) as xpool,
            tc.tile_pool(name="exp", bufs=8) as epool,
            tc.tile_pool(name="ys", bufs=3) as ypool,
            tc.tile_pool(name="rec", bufs=4) as rpool,
            tc.tile_pool(name="pp", bufs=2, space="PSUM") as pp,
            tc.tile_pool(name="ps", bufs=1, space="PSUM") as ps,
            tc.tile_pool(name="pu", bufs=1, space="PSUM") as pu,
            tc.tile_pool(name="po", bufs=1, space="PSUM") as po,
            tc.tile_pool(name="py", bufs=2, space="PSUM") as py,
        ):
            # ---- constants / weights resident in SBUF ----
            wq_s = cpool.tile([P, NE, D], BF, tag="wq")
            wk_s = cpool.tile([P, NE, D], BF, tag="wk")
            wv_s = cpool.tile([P, NE, D], BF, tag="wv")
            wo_s = cpool.tile([P, HL, E], BF, tag="wo")
            bqk_s = cpool.tile([P, 2 * HL], FP32, tag="bqk")
            bvr_s = cpool.tile([P, D], FP32, tag="bvr")
            mask_s = cpool.tile([P, 4, 512], BF, tag="mask")
            ones_s = cpool.tile([P, P], BF, tag="ones")

            wq_r = wq.rearrange("(t p) d -> p t d", p=P)
            wk_r = wk.rearrange("(t p) d -> p t d", p=P)
            wv_r = wv.rearrange("(t p) d -> p t d", p=P)
            xts0 = []
            for e in range(NE):
                xt = xpool.tile([P, 512], BF, tag="xt")
                xts0.append(xt)
            for q4 in range(4):
                s4 = slice(4 * q4, 4 * (q4 + 1))
                nc.sync.dma_start(wq_s[:, s4, :], wq_r[:, s4, :])
                nc.sync.dma_start(wk_s[:, s4, :], wk_r[:, s4, :])
                for e in range(4 * q4, 4 * q4 + 4):
                    nc.sync.dma_start(xts0[e][:], xT[P * e : P * (e + 1), 0:512])
            nc.sync.dma_start(bqk_s[:], bqk[:])
            nc.sync.dma_start(bvr_s[:], bvr[:])
            for q4 in range(4):
                s4 = slice(4 * q4, 4 * (q4 + 1))
                nc.sync.dma_start(wv_s[:, s4, :], wv_r[:, s4, :])
            nc.sync.dma_start(mask_s[:], maskT.rearrange("r p q -> p r q"))
            nc.sync.dma_start(wo_s[:], wo.rearrange("(t p) e -> p t e", p=P))
            nc.vector.memset(ones_s[:], 1.0)

            # persistent K^T [d, h, s] and V [s-tile, d] for the whole sequence
            KT_s = ppool.tile([P, HL, S], BF, tag="KT")
            V_s = ppool.tile([P, NE, D], BF, tag="V")

            def emit_norm(item):
                AOT_c, pu_s, po_s, ph = item
                rc = rpool.tile([P, 512], FP32, tag="rc")
                for qq in range(4):
                    qs = slice(P * qq, P * (qq + 1))
                    nc.vector.reciprocal(rc[:, qs], pu_s[:, qs])
                    nc.vector.tensor_mul(AOT_c[:, ph, qs], po_s[:, qs], rc[:, qs])

            def emit_out_proj(AOT_c, sc):
                # y[s, e] for chunk at sequence offset sc
                for st in range(4):
                    for n in range(4):
                        pyt = py.tile([P, 512], FP32, tag="py")
                        for h in range(HL):
                            nc.tensor.matmul(
                                pyt[:],
                                AOT_c[:, h, P * st : P * (st + 1)],
                                wo_s[:, h, 512 * n : 512 * (n + 1)],
                                start=(h == 0),
                                stop=(h == HL - 1),
                            )
                        ys = ypool.tile([P, 512], FP32, tag="ys")
                        nc.vector.tensor_copy(ys[:], pyt[:])
                        nc.sync.dma_start(
                            y_out[
                                sc + P * st : sc + P * (st + 1),
                                512 * n : 512 * (n + 1),
                            ],
                            ys[:],
                        )

            pending_proj = None
            norm_q = []
            for j in range(NC):  # s/q chunk of 512
                sj = 512 * j
                # ---- stream x^T for this chunk ----
                if j == 0:
                    xts = xts0
                else:
                    xts = []
                    for e in range(NE):
                        xt = xpool.tile([P, 512], BF, tag="xt")
                        nc.gpsimd.dma_start(
                            xt[:], xT[P * e : P * (e + 1), sj : sj + 512]
                        )
                        xts.append(xt)

                # deferred softmax normalizations run here: DVE is idle
                # during the QK projection phase
                while norm_q:
                    emit_norm(norm_q.pop(0))

                # ---- Q^T, K^T per head (fused e-loop): psum[d=128, s=512] ----
                QT_j = qtpool.tile([P, HL, 512], BF, tag="QT")
                for h in range(HL):
                    pq = pp.tile([P, 512], FP32, tag="pp", name="pqf")
                    pk = pp.tile([P, 512], FP32, tag="pp", name="pkf")
                    for e in range(NE):
                        nc.tensor.matmul(
                            pq[:],
                            wq_s[:, e, P * h : P * (h + 1)],
                            xts[e][:],
                            start=(e == 0),
                            stop=(e == NE - 1),
                        )
                        nc.tensor.matmul(
                            pk[:],
                            wk_s[:, e, P * h : P * (h + 1)],
                            xts[e][:],
                            start=(e == 0),
                            stop=(e == NE - 1),
                        )
                    nc.scalar.activation(
                        QT_j[:, h, :], pq[:], AF.Identity, bias=bqk_s[:, h : h + 1]
                    )
                    nc.scalar.activation(
                        KT_s[:, h, sj : sj + 512],
                        pk[:],
                        AF.Identity,
                        bias=bqk_s[:, HL + h : HL + h + 1],
                    )

                nc.sync.dma_start(
                    kT_out.rearrange("(h p) s -> p h s", p=P)[:, :, sj : sj + 512],
                    KT_s[:, :, sj : sj + 512],
                )

                # ---- V natural: psum[s=128, d=512] ----
                for st in range(4):
                    pv = pp.tile([P, 512], FP32, tag="pp", name="pvf")
                    for e in range(NE):
                        nc.tensor.matmul(
                            pv[:],
                            xts[e][:, P * st : P * (st + 1)],
                            wv_s[:, e, :],
                            start=(e == 0),
                            stop=(e == NE - 1),
                        )
                    nc.vector.tensor_add(V_s[:, 4 * j + st, :], pv[:], bvr_s[:])
                    nc.sync.dma_start(
                        v_out.rearrange("(t p) d -> p t d", p=P)[:, 4 * j + st, :],
                        V_s[:, 4 * j + st, :],
                    )

                # ---- deferred output projection for chunk j-1 ----
                if pending_proj is not None:
                    emit_out_proj(*pending_proj)
                    pending_proj = None

                # ---- attention for q-chunk j, all 4 heads ----
                AOT_j = aotpool.tile([P, HL, 512], BF, tag="AOT")
                nk = 4 * j + 4  # causal k-tiles
                norm_keep = 1 if j == NC - 1 else 9
                for h in range(HL):
                    if len(norm_q) > norm_keep:
                        emit_norm(norm_q.pop(0))
                    ets = []
                    psum_u = pu.tile([P, 512], FP32, tag="pu")
                    for pr in range(nk // 2):
                        pscr = ps.tile([P, 1024], FP32, tag="ps")
                        for half in range(2):
                            i = 2 * pr + half
                            nc.tensor.matmul(
                                pscr[:, 512 * half : 512 * (half + 1)],
                                KT_s[:, h, P * i : P * (i + 1)],
                                QT_j[:, h, :],
                                start=True,
                                stop=True,
                            )
                        et = epool.tile([P, 1024], BF, tag="et")
                        nc.scalar.activation(et[:], pscr[:], AF.Exp, scale=SCALE)
                        for half in range(2):
                            i = 2 * pr + half
                            eh = et[:, 512 * half : 512 * (half + 1)]
                            if i >= 4 * j:
                                nc.vector.tensor_mul(
                                    eh, eh, mask_s[:, i - 4 * j, :]
                                )
                            nc.tensor.matmul(
                                psum_u[:],
                                ones_s[:],
                                eh,
                                start=(i == 0),
                                stop=(i == nk - 1),
                            )
                            ets.append(eh)
                    psum_o = po.tile([P, 512], FP32, tag="po")
                    for i, eh in enumerate(ets):
                        nc.tensor.matmul(
                            psum_o[:],
                            V_s[:, i, P * h : P * (h + 1)],
                            eh,
                            start=(i == 0),
                            stop=(i == nk - 1),
                        )
                    us = rpool.tile([P, 512], FP32, tag="us")
                    nc.vector.tensor_copy(us[:], psum_u[:])
                    os_ = rpool.tile([P, 512], FP32, tag="os")
                    nc.vector.tensor_copy(os_[:], psum_o[:])
                    norm_q.append((AOT_j, us, os_, h))

                pending_proj = (AOT_j, sj)

            for item in norm_q:
                emit_norm(item)
            if pending_proj is not None:
                emit_out_proj(*pending_proj)

    return nc


_NC_CACHE = None


def _get_graph():
    global _NC_CACHE
    if _NC_CACHE is None:
        _NC_CACHE = build_graph()
    return _NC_CACHE


def _prep_in_maps(x, Wq, bq, Wk, bk, Wv, bv, Wo):
    maskT = np.zeros((4, P, 512), dtype=BF16)
    kk = np.arange(P)[:, None]
    qq = np.arange(512)[None, :]
    for r in range(4):
        maskT[r] = (qq >= kk + P * r).astype(BF16)

    xTb = [np.ascontiguousarray(x[b].T).astype(BF16) for b in range(B)]
    in_maps = []
    for c in range(8):
        b, g = divmod(c, 4)
        sl = slice(D * g, D * (g + 1))
        bqk = np.concatenate(
            [bq[sl].reshape(HL, P).T, bk[sl].reshape(HL, P).T], axis=1
        ).astype(np.float32)
        in_maps.append(
            {
                "xT": xTb[b],
                "wq": np.ascontiguousarray(Wq[:, sl]).astype(BF16),
                "wk": np.ascontiguousarray(Wk[:, sl]).astype(BF16),
                "wv": np.ascontiguousarray(Wv[:, sl]).astype(BF16),
                "wo": np.ascontiguousarray(Wo[sl, :]).astype(BF16),
                "bqk": np.ascontiguousarray(bqk),
                "bvr": np.ascontiguousarray(
                    np.broadcast_to(bv[sl].astype(np.float32), (P, D))
                ),
                "maskT": maskT,
            }
        )
    return in_maps


def run(inputs, trace=False):
    x = np.asarray(inputs["x"], dtype=np.float32)
    Wq = np.asarray(inputs["Wq"], dtype=np.float32)
    bq = np.asarray(inputs["bq"], dtype=np.float32)
    Wk = np.asarray(inputs["Wk"], dtype=np.float32)
    bk = np.asarray(inputs["bk"], dtype=np.float32)
    Wv = np.asarray(inputs["Wv"], dtype=np.float32)
    bv = np.asarray(inputs["bv"], dtype=np.float32)
    Wo = np.asarray(inputs["Wo"], dtype=np.float32)
    bo = np.asarray(inputs["bo"], dtype=np.float32)

    nc = _get_graph()
    in_maps = _prep_in_maps(x, Wq, bq, Wk, bk, Wv, bv, Wo)
    res = run_bass_kernel_spmd(nc, in_maps, core_ids=list(range(8)), trace=trace)

    y = np.zeros((B, S, E), dtype=np.float32)
    cached_kv = np.zeros((B, 2, H, S, DH), dtype=np.float32)
    for c in range(8):
        b, g = divmod(c, 4)
        out = res.results[c]
        y[b] += out["y"]
        kT = np.asarray(out["kT"], dtype=np.float32)  # [D, S]
        v = np.asarray(out["v"], dtype=np.float32)  # [S, D]
        for h in range(HL):
            cached_kv[b, 0, HL * g + h] = kT[P * h : P * (h + 1), :].T
            cached_kv[b, 1, HL * g + h] = v[:, P * h : P * (h + 1)]
    y += bo[None, None, :]
    return (y, cached_kv), res


def kernel(**inputs):
    (y, cached_kv), _ = run(inputs, trace=False)
    return y, cached_kv


# revision 20
# speedup vs baseline: 1.2291x; 1.2291x over previous
"""Causal MHA (B=2, S=2048, E=2048, H=16, Dh=128) on 8 trn2 NeuronCores.

Sharding (megatron-style, all host-side): core c -> batch b=c//4, head-group
g=c%4 (4 heads). Each core computes Q/K/V for its 4 heads from the full x[b],
causal attention, and a partial output projection y_partial = attn_out @
Wo[rows of g]. Host sums the 4 partials per batch (+bo) and reassembles the
kv cache. All 8 cores run one identical SPMD graph; no collectives.

Device layout trick: x is fed pre-transposed [E, S] so the projections
produce Q^T/K^T per head ([d=128 partitions, s free]) and V natural [s, d]
directly. Scores are computed transposed ([k, q]); exp runs on ScalarE with
the 1/sqrt(Dh) scale folded in; the causal mask is a 0/1 multiply; the
softmax denominator is a ones-matmul on the TensorEngine (partition-dim sum,
replicated on all partitions); attn@V consumes exp^T as the matmul rhs with
V as lhsT; the output projection consumes attn_out^T as lhsT. Zero on-chip
transposes.
"""

import sys

import numpy as np

try:
    import concourse.bass as bass
except ImportError:
    sys.path.insert(0, "/opt/trn_rl_repo")
    import concourse.bass as bass

import bass_rust
import ml_dtypes
import concourse.mybir as mybir
import concourse.tile as tile
from concourse.vector_clock import ScopedClock
from concourse.bass_utils import run_bass_kernel_spmd

BF16 = ml_dtypes.bfloat16
B, S, E, H = 2, 2048, 2048, 16
DH = 128
P = 128
D = 512  # per-core head-dim slice (4 heads)
HL = 4  # local heads
NE = E // P  # 16 e-tiles
NC = S // 512  # 4 s/q chunks
SCALE = 1.0 / float(np.sqrt(DH))
FP32 = mybir.dt.float32
BF = mybir.dt.bfloat16


def _patched_drain_and_barrier(self, tick_clock, wait_clock):
    # Walrus CoreV3 lowers Drain to a TPB_CTRL encoding with a single
    # sync-wait slot; Tile's stock exit puts every outstanding proc's wait on
    # one drain and the compile fails with "Too many sync wait commands".
    # Keep one wait on the drain and chain the rest through SP nops.
    drain_inst = self.nc.sync.drain()
    wait_clock.add_sem_waits(
        drain_inst.ins, ScopedClock({None: tick_clock.global_clock})
    )
    si = drain_inst.ins.sync_info
    waits = list(si.on_wait) if si is not None else []
    if len(waits) > 1:
        si.on_wait = waits[:1]
        drain_inst.ins.sync_info = si
        for w in waits[1:]:
            extra = self.nc.sync.nop(nofuse=True, hint="drain_wait_chain")
            extra.ins.sync_info = bass_rust.SyncInfo(on_wait=[w], on_update=[])

    self.nc.all_engine_barrier()
    assert self.sems is not None
    popped = self.nc._tile_sem_poison_stack.pop()
    assert popped is self._sem_poison
    self.nc.clear_and_free_semaphores(list(self.sems.allocated().values()))
    self.nc.all_engine_barrier()


tile.TileContext._drain_and_barrier = _patched_drain_and_barrier

_orig_lower_ordered = tile.TileContext._lower_ordered_insts
_split_ctr = [0]


def _split_waits_lower(self, ordered):
    # Same walrus limitation as above, but for body instructions: each
    # instruction may carry at most one sync wait. Hoist extras onto
    # same-engine NoOps placed immediately before the instruction.
    for bb_name, insts in ordered.items():
        new = []
        for inst in insts:
            si = inst.sync_info
            if si is not None and len(si.on_wait) > 1:
                waits = list(si.on_wait)
                for w in waits[:-1]:
                    _split_ctr[0] += 1
                    nop = mybir.InstNoOp(name=f"waitsplit-{_split_ctr[0]}")
                    nop.engine = inst.engine
                    nop.sync_info = bass_rust.SyncInfo(on_wait=[w], on_update=[])
                    new.append(nop)
                si.on_wait = waits[-1:]
                inst.sync_info = si
            new.append(inst)
        ordered[bb_name] = new
    return _orig_lower_ordered(self, ordered)


tile.TileContext._lower_ordered_insts = _split_waits_lower

AF = mybir.ActivationFunctionType


def build_graph():
    nc = bass.Bass(trn_type="TRN2")

    xT = nc.dram_tensor("xT", [E, S], BF, kind="ExternalInput")
    wq = nc.dram_tensor("wq", [E, D], BF, kind="ExternalInput")
    wk = nc.dram_tensor("wk", [E, D], BF, kind="ExternalInput")
    wv = nc.dram_tensor("wv", [E, D], BF, kind="ExternalInput")
    wo = nc.dram_tensor("wo", [D, E], BF, kind="ExternalInput")
    bqk = nc.dram_tensor("bqk", [P, 2 * HL], FP32, kind="ExternalInput")
    bvr = nc.dram_tensor("bvr", [P, D], FP32, kind="ExternalInput")
    maskT = nc.dram_tensor("maskT", [4, P, 512], BF, kind="ExternalInput")

    y_out = nc.dram_tensor("y", [S, E], FP32, kind="ExternalOutput")
    kT_out = nc.dram_tensor("kT", [D, S], BF, kind="ExternalOutput")
    v_out = nc.dram_tensor("v", [S, D], BF, kind="ExternalOutput")

    with tile.TileContext(nc) as tc:
        with (
            tc.tile_pool(name="const", bufs=1) as cpool,
            tc.tile_pool(name="persist", bufs=1) as ppool,
            tc.tile_pool(name="qt", bufs=2) as qtpool,
            tc.tile_pool(name="aot", bufs=2) as aotpool,
            tc.tile_pool(name="xs", bufs=20) as xpool,
            tc.tile_pool(name="exp", bufs=8) as epool,
            tc.tile_pool(name="ys", bufs=3) as ypool,
            tc.tile_pool(name="rec", bufs=4) as rpool,
            tc.tile_pool(name="pp", bufs=2, space="PSUM") as pp,
            tc.tile_pool(name="ps", bufs=2, space="PSUM") as ps,
            tc.tile_pool(name="pu", bufs=1, space="PSUM") as pu,
            tc.tile_pool(name="po", bufs=1, space="PSUM") as po,
            tc.tile_pool(name="py", bufs=2, space="PSUM") as py,
        ):
            # ---- constants / weights resident in SBUF ----
            wq_s = cpool.tile([P, NE, D], BF, tag="wq")
            wk_s = cpool.tile([P, NE, D], BF, tag="wk")
            wv_s = cpool.tile([P, NE, D], BF, tag="wv")
            wo_s = cpool.tile([P, HL, E], BF, tag="wo")
            bqk_s = cpool.tile([P, 2 * HL], FP32, tag="bqk")
            bvr_s = cpool.tile([P, D], FP32, tag="bvr")
            mask_s = cpool.tile([P, 4, 512], BF, tag="mask")
            ones_s = cpool.tile([P, P], BF, tag="ones")

            wq_r = wq.rearrange("(t p) d -> p t d", p=P)
            wk_r = wk.rearrange("(t p) d -> p t d", p=P)
            wv_r = wv.rearrange("(t p) d -> p t d", p=P)
            xts0 = []
            for e in range(NE):
                xt = xpool.tile([P, 512], BF, tag="xt")
                xts0.append(xt)
            for q4 in range(4):
                s4 = slice(4 * q4, 4 * (q4 + 1))
                nc.sync.dma_start(wq_s[:, s4, :], wq_r[:, s4, :])
                nc.sync.dma_start(wk_s[:, s4, :], wk_r[:, s4, :])
                for e in range(4 * q4, 4 * q4 + 4):
                    nc.sync.dma_start(xts0[e][:], xT[P * e : P * (e + 1), 0:512])
            nc.sync.dma_start(bqk_s[:], bqk[:])
            nc.sync.dma_start(bvr_s[:], bvr[:])
            for q4 in range(4):
                s4 = slice(4 * q4, 4 * (q4 + 1))
                nc.sync.dma_start(wv_s[:, s4, :], wv_r[:, s4, :])
            nc.sync.dma_start(mask_s[:], maskT.rearrange("r p q -> p r q"))
            nc.sync.dma_start(wo_s[:], wo.rearrange("(t p) e -> p t e", p=P))
            nc.vector.memset(ones_s[:], 1.0)

            # persistent K^T [d, h, s] and V [s-tile, d] for the whole sequence
            KT_s = ppool.tile([P, HL, S], BF, tag="KT")
            V_s = ppool.tile([P, NE, D], BF, tag="V")

            def emit_norm(item):
                AOT_c, pu_s, po_s, ph = item
                rc = rpool.tile([P, 512], FP32, tag="rc")
                for qq in range(4):
                    qs = slice(P * qq, P * (qq + 1))
                    nc.vector.reciprocal(rc[:, qs], pu_s[:, qs])
                    nc.vector.tensor_mul(AOT_c[:, ph, qs], po_s[:, qs], rc[:, qs])

            def emit_out_proj(AOT_c, sc):
                # y[s, e] for chunk at sequence offset sc
                for st in range(4):
                    for n in range(4):
                        pyt = py.tile([P, 512], FP32, tag="py")
                        for h in range(HL):
                            nc.tensor.matmul(
                                pyt[:],
                                AOT_c[:, h, P * st : P * (st + 1)],
                                wo_s[:, h, 512 * n : 512 * (n + 1)],
                                start=(h == 0),
                                stop=(h == HL - 1),
                            )
                        ys = ypool.tile([P, 512], FP32, tag="ys")
                        nc.vector.tensor_copy(ys[:], pyt[:])
                        nc.sync.dma_start(
                            y_out[
                                sc + P * st : sc + P * (st + 1),
                                512 * n : 512 * (n + 1),
                            ],
                            ys[:],
                        )

            pending_proj = None
            norm_q = []
            for j in range(NC):  # s/q chunk of 512
                sj = 512 * j
                # ---- stream x^T for this chunk ----
                if j == 0:
                    xts = xts0
                else:
                    xts = []
                    for e in range(NE):
                        xt = xpool.tile([P, 512], BF, tag="xt")
                        nc.gpsimd.dma_start(
                            xt[:], xT[P * e : P * (e + 1), sj : sj + 512]
                        )
                        xts.append(xt)

                # deferred softmax normalizations run here: DVE is idle
                # during the QK projection phase
                while norm_q:
                    emit_norm(norm_q.pop(0))

                # ---- Q^T, K^T per head (fused e-loop): psum[d=128, s=512] ----
                QT_j = qtpool.tile([P, HL, 512], BF, tag="QT")
                for h in range(HL):
                    pq = pp.tile([P, 512], FP32, tag="pp", name="pqf")
                    pk = pp.tile([P, 512], FP32, tag="pp", name="pkf")
                    for e in range(NE):
                        nc.tensor.matmul(
                            pq[:],
                            wq_s[:, e, P * h : P * (h + 1)],
                            xts[e][:],
                            start=(e == 0),
                            stop=(e == NE - 1),
                        )
                        nc.tensor.matmul(
                            pk[:],
                            wk_s[:, e, P * h : P * (h + 1)],
                            xts[e][:],
                            start=(e == 0),
                            stop=(e == NE - 1),
                        )
                    nc.scalar.activation(
                        QT_j[:, h, :], pq[:], AF.Identity, bias=bqk_s[:, h : h + 1]
                    )
                    nc.scalar.activation(
                        KT_s[:, h, sj : sj + 512],
                        pk[:],
                        AF.Identity,
                        bias=bqk_s[:, HL + h : HL + h + 1],
                    )

                nc.sync.dma_start(
                    kT_out.rearrange("(h p) s -> p h s", p=P)[:, :, sj : sj + 512],
                    KT_s[:, :, sj : sj + 512],
                )

                # ---- V natural: psum[s=128, d=512] ----
                for st in range(4):
                    pv = pp.tile([P, 512], FP32, tag="pp", name="pvf")
                    for e in range(NE):
                        nc.tensor.matmul(
                            pv[:],
                            xts[e][:, P * st : P * (st + 1)],
                            wv_s[:, e, :],
                            start=(e == 0),
                            stop=(e == NE - 1),
                        )
                    nc.vector.tensor_add(V_s[:, 4 * j + st, :], pv[:], bvr_s[:])
                    nc.sync.dma_start(
                        v_out.rearrange("(t p) d -> p t d", p=P)[:, 4 * j + st, :],
                        V_s[:, 4 * j + st, :],
                    )

                # ---- deferred output projection for chunk j-1 ----
                if pending_proj is not None:
                    emit_out_proj(*pending_proj)
                    pending_proj = None

                # ---- attention for q-chunk j, all 4 heads ----
                AOT_j = aotpool.tile([P, HL, 512], BF, tag="AOT")
                nk = 4 * j + 4  # causal k-tiles
                norm_keep = 1 if j == NC - 1 else 9
                for h in range(HL):
                    if len(norm_q) > norm_keep:
                        emit_norm(norm_q.pop(0))
                    ets = []
                    psum_u = pu.tile([P, 512], FP32, tag="pu")
                    for i in range(nk):
                        pscr = ps.tile([P, 512], FP32, tag="ps")
                        nc.tensor.matmul(
                            pscr[:],
                            KT_s[:, h, P * i : P * (i + 1)],
                            QT_j[:, h, :],
                            start=True,
                            stop=True,
                        )
                        et = epool.tile([P, 512], BF, tag="et")
                        nc.scalar.activation(et[:], pscr[:], AF.Exp, scale=SCALE)
                        if i >= 4 * j:
                            nc.vector.tensor_mul(
                                et[:], et[:], mask_s[:, i - 4 * j, :]
                            )
                        nc.tensor.matmul(
                            psum_u[:],
                            ones_s[:],
                            et[:],
                            start=(i == 0),
                            stop=(i == nk - 1),
                        )
                        ets.append(et)
                    psum_o = po.tile([P, 512], FP32, tag="po")
                    for i, eh in enumerate(ets):
                        nc.tensor.matmul(
                            psum_o[:],
                            V_s[:, i, P * h : P * (h + 1)],
                            eh,
                            start=(i == 0),
                            stop=(i == nk - 1),
                        )
                    us = rpool.tile([P, 512], FP32, tag="us")
                    nc.vector.tensor_copy(us[:], psum_u[:])
                    os_ = rpool.tile([P, 512], FP32, tag="os")
                    nc.vector.tensor_copy(os_[:], psum_o[:])
                    norm_q.append((AOT_j, us, os_, h))

                pending_proj = (AOT_j, sj)

            for item in norm_q:
                emit_norm(item)
            if pending_proj is not None:
                emit_out_proj(*pending_proj)

    return nc


_NC_CACHE = None


def _get_graph():
    global _NC_CACHE
    if _NC_CACHE is None:
        _NC_CACHE = build_graph()
    return _NC_CACHE


def _prep_in_maps(x, Wq, bq, Wk, bk, Wv, bv, Wo):
    maskT = np.zeros((4, P, 512), dtype=BF16)
    kk = np.arange(P)[:, None]
    qq = np.arange(512)[None, :]
    for r in range(4):
        maskT[r] = (qq >= kk + P * r).astype(BF16)

    xTb = [np.ascontiguousarray(x[b].T).astype(BF16) for b in range(B)]
    in_maps = []
    for c in range(8):
        b, g = divmod(c, 4)
        sl = slice(D * g, D * (g + 1))
        bqk = np.concatenate(
            [bq[sl].reshape(HL, P).T, bk[sl].reshape(HL, P).T], axis=1
        ).astype(np.float32)
        in_maps.append(
            {
                "xT": xTb[b],
                "wq": np.ascontiguousarray(Wq[:, sl]).astype(BF16),
                "wk": np.ascontiguousarray(Wk[:, sl]).astype(BF16),
                "wv": np.ascontiguousarray(Wv[:, sl]).astype(BF16),
                "wo": np.ascontiguousarray(Wo[sl, :]).astype(BF16),
                "bqk": np.ascontiguousarray(bqk),
                "bvr": np.ascontiguousarray(
                    np.broadcast_to(bv[sl].astype(np.float32), (P, D))
                ),
                "maskT": maskT,
            }
        )
    return in_maps


def run(inputs, trace=False):
    x = np.asarray(inputs["x"], dtype=np.float32)
    Wq = np.asarray(inputs["Wq"], dtype=np.float32)
    bq = np.asarray(inputs["bq"], dtype=np.float32)
    Wk = np.asarray(inputs["Wk"], dtype=np.float32)
    bk = np.asarray(inputs["bk"], dtype=np.float32)
    Wv = np.asarray(inputs["Wv"], dtype=np.float32)
    bv = np.asarray(inputs["bv"], dtype=np.float32)
    Wo = np.asarray(inputs["Wo"], dtype=np.float32)
    bo = np.asarray(inputs["bo"], dtype=np.float32)

    nc = _get_graph()
    in_maps = _prep_in_maps(x, Wq, bq, Wk, bk, Wv, bv, Wo)
    res = run_bass_kernel_spmd(nc, in_maps, core_ids=list(range(8)), trace=trace)

    y = np.zeros((B, S, E), dtype=np.float32)
    cached_kv = np.zeros((B, 2, H, S, DH), dtype=np.float32)
    for c in range(8):
        b, g = divmod(c, 4)
        out = res.results[c]
        y[b] += out["y"]
        kT = np.asarray(out["kT"], dtype=np.float32)  # [D, S]
        v = np.asarray(out["v"], dtype=np.float32)  # [S, D]
        for h in range(HL):
            cached_kv[b, 0, HL * g + h] = kT[P * h : P * (h + 1), :].T
            cached_kv[b, 1, HL * g + h] = v[:, P * h : P * (h + 1)]
    y += bo[None, None, :]
    return (y, cached_kv), res


def kernel(**inputs):
    (y, cached_kv), _ = run(inputs, trace=False)
    return y, cached_kv
